# revision 9
# baseline (speedup 1.0000x reference)
"""Trainium2 Bass kernel for NeuralConnectionMatrix.

out[i, j] = W2 . relu(R[i, :] + L[j, :] + b1) + b2
  where L = fv @ W1[:, :F].T  (depends on j), R = fv @ W1[:, F:].T (depends on i)

Sharding (8 cores): 2 i-groups x 4 j-groups. Each core computes a
[1024 j, 2048 i] transposed slab:
  - partitions = j (8 blocks of 128), free dim = i (2048)
  - per k: t_k = w2k*relu(x_k) computed as (w2k*R_bcast + w2k*Lb_bias) max/min 0
    (min-0 trick bakes negative w2k signs in host-side); relus split
    across VectorE (tensor_scalar) and ScalarE (activation relu)
  - pairs of t_k are pre-summed on GpSimd/VectorE (b2 folded into the
    first merge) to cut TensorE matmul count
  - PE accumulates the resulting slices into PSUM via identity matmuls
  - ACT drains PSUM -> SBUF, DMA to DRAM
Host precomputes L/R (tiny GEMMs), replicates R across partitions, and
transposes the per-core output slabs back into the full [4096, 4096] array.
"""

import numpy as np

import concourse.bass as bass
import concourse.bacc as bacc
import concourse.mybir as mybir
from concourse.tile import TileContext
from concourse.bass_utils import run_bass_kernel_spmd

N = 4096
F = 3
H = 16
NCORES = 8
IG, JG = 2, 4            # core grid over (i, j)
FI = N // IG             # free-dim (i) extent per core: 2048
PJ = N // JG             # partition-dim (j) extent per core: 1024
NJB = PJ // 128          # j blocks per core: 8
NMM = FI // 512          # matmuls per slice per j-block (PSUM bank = 512 f32)

FP16 = mybir.dt.float16
FP32 = mybir.dt.float32
ALU = mybir.AluOpType


def _plan(w2):
    """Assign the 16 k-slices to engines and merge pairs.

    Returns (act_ks, slices_spec) where slices_spec entries are
    ("s", k) or ("p", ka, kb, eng, fold_b2). Ordered so consumption
    follows production (singles of early k's first, then pairs).
    """
    pos = [int(k) for k in range(H) if w2[k] >= 0]
    neg = [int(k) for k in range(H) if w2[k] < 0]
    act_n = min(4, len(pos))
    act_ks = pos[:act_n]
    dve_ks = pos[act_n:] + neg

    d = list(dve_ks)
    a = list(act_ks)
    singles = d[:4] + a[:-2] if len(a) >= 2 else d[:4] + a
    rest = d[4:]
    pairs = []
    while len(rest) >= 2:
        eng = "gps" if len(pairs) < 3 else "dve"
        pairs.append((rest.pop(0), rest.pop(0), eng))
    if rest:
        singles.append(rest.pop(0))
    if len(a) >= 2:
        pairs.insert(min(3, len(pairs)), (a[-2], a[-1], "gps"))
    if not pairs and len(singles) >= 2:
        pairs.append((singles.pop(0), singles.pop(0), "dve"))

    # b2 folds into a scalar_tensor_tensor merge, which only DVE supports
    b2_pi = next(
        (pi for pi, p in enumerate(pairs) if p[2] == "dve"), None
    )
    if b2_pi is None and pairs:
        pairs[0] = (pairs[0][0], pairs[0][1], "dve")
        b2_pi = 0
    slices_spec = [("s", k) for k in singles]
    for pi, (ka, kb, eng) in enumerate(pairs):
        slices_spec.append(("p", ka, kb, eng, pi == b2_pi))
    return act_ks, slices_spec


def build_bass(min_ks, act_ks, slices_spec):
    nc = bacc.Bacc()
    rb = nc.dram_tensor("rb", [H, 128, FI], FP16, kind="ExternalInput")
    lbt = nc.dram_tensor("lbt", [NJB, 128, H], FP32, kind="ExternalInput")
    ident = nc.dram_tensor("ident", [128, 128], FP16, kind="ExternalInput")
    b2t = nc.dram_tensor("b2t", [128, 1], FP32, kind="ExternalInput")
    outT = nc.dram_tensor("outT", [PJ, FI], FP32, kind="ExternalOutput")

    with TileContext(nc) as tc:
        with (
            tc.tile_pool(name="const", bufs=1) as cpool,
            tc.tile_pool(name="t", bufs=10) as tpool,
            tc.tile_pool(name="u", bufs=6) as upool,
            tc.tile_pool(name="o", bufs=2) as opool,
            tc.tile_pool(name="ps", bufs=2, space="PSUM") as pspool,
        ):
            id_t = cpool.tile([128, 128], FP16, tag="ident")
            nc.gpsimd.dma_start(out=id_t, in_=ident[:, :])
            lb_all = cpool.tile([128, NJB * H], FP32, tag="lball")
            nc.gpsimd.dma_start(
                out=lb_all.rearrange("p (b k) -> p b k", b=NJB),
                in_=lbt.rearrange("b p k -> p b k"),
            )
            b2_sb = cpool.tile([128, 1], FP32, tag="b2")
            nc.gpsimd.dma_start(out=b2_sb, in_=b2t[:, :])
            rbs = []
            for k in range(H):
                rt = cpool.tile([128, FI], FP16, tag=f"rb{k}")
                nc.sync.dma_start(out=rt, in_=rb[k])
                rbs.append(rt)

            for jb in range(NJB):
                ps = pspool.tile([128, FI], FP32, tag="ps")

                def relu(k):
                    t = tpool.tile([128, FI], FP16, tag="t")
                    lb_ap = lb_all[:, jb * H + k : jb * H + k + 1]
                    if k in act_ks:
                        nc.scalar.activation(
                            t, rbs[k], mybir.ActivationFunctionType.Relu,
                            bias=lb_ap, scale=1.0,
                        )
                    else:
                        op1 = ALU.min if k in min_ks else ALU.max
                        nc.vector.tensor_scalar(
                            out=t, in0=rbs[k],
                            scalar1=lb_ap, scalar2=0.0,
                            op0=ALU.add, op1=op1,
                        )
                    return t

                nsl = len(slices_spec)
                for si, spec in enumerate(slices_spec):
                    if spec[0] == "s":
                        sl = relu(spec[1])
                    else:
                        _, ka, kb, eng_name, fold_b2 = spec
                        ta, tb = relu(ka), relu(kb)
                        sl = upool.tile([128, FI], FP16, tag="u")
                        eng = nc.gpsimd if eng_name == "gps" else nc.vector
                        if fold_b2:
                            eng.scalar_tensor_tensor(
                                out=sl, in0=ta, scalar=b2_sb[:, 0:1],
                                in1=tb, op0=ALU.add, op1=ALU.add,
                            )
                        else:
                            eng.tensor_add(out=sl, in0=ta, in1=tb)
                    for nb in range(NMM):
                        nc.tensor.matmul(
                            ps[:, nb * 512 : (nb + 1) * 512],
                            id_t,
                            sl[:, nb * 512 : (nb + 1) * 512],
                            start=(si == 0), stop=(si == nsl - 1),
                        )
                ot = opool.tile([128, FI], FP32, tag="o")
                nc.scalar.copy(ot, ps)
                nc.sync.dma_start(
                    out=outT[jb * 128 : (jb + 1) * 128, :], in_=ot
                )
    nc.finalize()
    return nc


def _prep(feature_vectors, W1, b1, W2, b2):
    fv = np.asarray(feature_vectors, dtype=np.float32)
    W1 = np.asarray(W1, dtype=np.float32)
    b1 = np.asarray(b1, dtype=np.float32)
    W2 = np.asarray(W2, dtype=np.float32)
    b2 = np.asarray(b2, dtype=np.float32)

    L = fv @ W1[:, :F].T + b1        # [N, H], j side (bias, on partitions)
    R = fv @ W1[:, F:].T             # [N, H], i side (free dim)
    w2 = W2[0]                       # [H]
    b2v = float(b2[0])

    # Fold w2 into both operands; negative w2k handled with min-0 trick.
    Rs = R * w2[None, :]
    Ls = L * w2[None, :]

    min_ks = tuple(int(k) for k in range(H) if w2[k] < 0)
    act_ks, slices_spec = _plan(w2)
    nc = build_bass(min_ks, act_ks, slices_spec)

    ident = np.eye(128, dtype=np.float16)
    b2arr = np.full((128, 1), b2v, dtype=np.float32)
    in_maps = []
    for c in range(NCORES):
        ig, jg = divmod(c, JG)
        isl = slice(ig * FI, (ig + 1) * FI)
        jsl = slice(jg * PJ, (jg + 1) * PJ)
        base = Rs[isl, :].T.astype(np.float16)          # [H, FI]
        rb_c = np.ascontiguousarray(
            np.broadcast_to(base[:, None, :], (H, 128, FI))
        )
        lbt_c = np.ascontiguousarray(Ls[jsl, :].reshape(NJB, 128, H))
        in_maps.append(
            {"rb": rb_c, "lbt": lbt_c, "ident": ident, "b2t": b2arr}
        )
    return nc, in_maps


def _gather(res):
    out = np.empty((N, N), dtype=np.float32)
    for c in range(NCORES):
        ig, jg = divmod(c, JG)
        out[ig * FI : (ig + 1) * FI, jg * PJ : (jg + 1) * PJ] = (
            res.results[c]["outT"].T
        )
    return out


def kernel(feature_vectors, W1, b1, W2, b2):
    nc, in_maps = _prep(feature_vectors, W1, b1, W2, b2)
    res = run_bass_kernel_spmd(nc, in_maps, core_ids=list(range(NCORES)))
    return _gather(res)


# revision 10
# speedup vs baseline: 1.8196x; 1.8196x over previous
"""Trainium2 Bass kernel for NeuralConnectionMatrix.

out[i, j] = W2 . relu(R[i, :] + L[j, :] + b1) + b2
  where L = fv @ W1[:, :F].T  (depends on j), R = fv @ W1[:, F:].T (depends on i)

Sharding (8 cores): 2 i-groups x 4 j-groups. Each core computes a
[1024 j, 2048 i] transposed slab:
  - partitions = j (8 blocks of 128), free dim = i (2048)
  - per k: t_k = w2k*relu(x_k) computed as (w2k*R_bcast + w2k*Lb_bias) max/min 0
    (min-0 trick bakes negative w2k signs in host-side); relus split
    across VectorE (tensor_scalar) and ScalarE (activation relu)
  - pairs of t_k are pre-summed on GpSimd/VectorE (b2 folded into the
    first merge) to cut TensorE matmul count
  - PE accumulates the resulting slices into PSUM via identity matmuls
  - ACT drains PSUM -> SBUF, DMA to DRAM
Host precomputes L/R (tiny GEMMs), replicates R across partitions, and
transposes the per-core output slabs back into the full [4096, 4096] array.
"""

import numpy as np

import concourse.bass as bass
import concourse.bacc as bacc
import concourse.mybir as mybir
from concourse.tile import TileContext
from concourse.bass_utils import run_bass_kernel_spmd

N = 4096
F = 3
H = 16
NCORES = 8
IG, JG = 2, 4            # core grid over (i, j)
FI = N // IG             # free-dim (i) extent per core: 2048
PJ = N // JG             # partition-dim (j) extent per core: 1024
NJB = PJ // 128          # j blocks per core: 8
NMM = FI // 512          # matmuls per slice per j-block (PSUM bank = 512 f32)

FP16 = mybir.dt.float16
FP32 = mybir.dt.float32
ALU = mybir.AluOpType


def _plan(w2, act_n=4, n_pairs=2):
    """Assign the 16 k-slices to engines and merge pairs.

    Returns (act_ks, slices_spec): entries ("s", k) or
    ("p", ka, kb, fold_b2); pairs run on DVE (gpsimd 2-input ops are
    slow, ScalarE can't). b2 folds into the first pair via
    scalar_tensor_tensor. Ordered roughly by production.
    """
    pos = [int(k) for k in range(H) if w2[k] >= 0]
    neg = [int(k) for k in range(H) if w2[k] < 0]
    act_n = min(act_n, len(pos))
    act_ks = pos[:act_n]
    d = pos[act_n:] + neg
    a = list(act_ks)

    pairs = []
    while len(pairs) < n_pairs and len(d) >= 2:
        pairs.append((d.pop(0), d.pop(0)))

    slices_spec = []
    # interleave ACT singles among DVE singles; pairs in the middle
    mix = []
    di, ai = 0, 0
    for k in range(len(d) + len(a)):
        if ai < len(a) and (di >= len(d) or k % 3 == 2):
            mix.append(("s", a[ai])); ai += 1
        else:
            mix.append(("s", d[di])); di += 1
    mid = len(mix) // 2
    slices_spec = mix[:mid]
    for pi, (ka, kb) in enumerate(pairs):
        slices_spec.append(("p", ka, kb, pi == 0))
    slices_spec += mix[mid:]
    if not pairs:
        raise ValueError("need at least one pair to fold b2")
    return act_ks, slices_spec


def _rb_order(slices_spec):
    """k indices in consumption order (for DMA issue order)."""
    order = []
    for spec in slices_spec:
        if spec[0] == "s":
            order.append(spec[1])
        else:
            order.extend([spec[1], spec[2]])
    return order


def build_bass(min_ks, act_ks, slices_spec):
    nc = bacc.Bacc()
    rb = nc.dram_tensor("rb", [H, 128, FI], FP16, kind="ExternalInput")
    lbt = nc.dram_tensor("lbt", [NJB, 128, H], FP32, kind="ExternalInput")
    ident = nc.dram_tensor("ident", [128, 128], FP16, kind="ExternalInput")
    b2t = nc.dram_tensor("b2t", [128, 1], FP32, kind="ExternalInput")
    outT = nc.dram_tensor("outT", [PJ, FI], FP32, kind="ExternalOutput")

    with TileContext(nc) as tc:
        with (
            tc.tile_pool(name="const", bufs=1) as cpool,
            tc.tile_pool(name="t", bufs=10) as tpool,
            tc.tile_pool(name="u", bufs=6) as upool,
            tc.tile_pool(name="o", bufs=2) as opool,
            tc.tile_pool(name="ps", bufs=2, space="PSUM") as pspool,
        ):
            id_t = cpool.tile([128, 128], FP16, tag="ident")
            nc.gpsimd.dma_start(out=id_t, in_=ident[:, :])
            lb_all = cpool.tile([128, NJB * H], FP32, tag="lball")
            nc.gpsimd.dma_start(
                out=lb_all.rearrange("p (b k) -> p b k", b=NJB),
                in_=lbt.rearrange("b p k -> p b k"),
            )
            b2_sb = cpool.tile([128, 1], FP32, tag="b2")
            nc.gpsimd.dma_start(out=b2_sb, in_=b2t[:, :])
            rbs = {}
            for k in _rb_order(slices_spec):
                rt = cpool.tile([128, FI], FP16, tag=f"rb{k}")
                nc.sync.dma_start(out=rt, in_=rb[k])
                rbs[k] = rt

            for jb in range(NJB):
                ps = pspool.tile([128, FI], FP32, tag="ps")

                def relu(k):
                    t = tpool.tile([128, FI], FP16, tag="t")
                    lb_ap = lb_all[:, jb * H + k : jb * H + k + 1]
                    if k in act_ks:
                        nc.scalar.activation(
                            t, rbs[k], mybir.ActivationFunctionType.Relu,
                            bias=lb_ap, scale=1.0,
                        )
                    else:
                        op1 = ALU.min if k in min_ks else ALU.max
                        nc.vector.tensor_scalar(
                            out=t, in0=rbs[k],
                            scalar1=lb_ap, scalar2=0.0,
                            op0=ALU.add, op1=op1,
                        )
                    return t

                nsl = len(slices_spec)
                for si, spec in enumerate(slices_spec):
                    if spec[0] == "s":
                        sl = relu(spec[1])
                    else:
                        _, ka, kb, fold_b2 = spec
                        ta, tb = relu(ka), relu(kb)
                        sl = upool.tile([128, FI], FP16, tag="u")
                        if fold_b2:
                            nc.vector.scalar_tensor_tensor(
                                out=sl, in0=ta, scalar=b2_sb[:, 0:1],
                                in1=tb, op0=ALU.add, op1=ALU.add,
                            )
                        else:
                            nc.vector.tensor_add(out=sl, in0=ta, in1=tb)
                    for nb in range(NMM):
                        nc.tensor.matmul(
                            ps[:, nb * 512 : (nb + 1) * 512],
                            id_t,
                            sl[:, nb * 512 : (nb + 1) * 512],
                            start=(si == 0), stop=(si == nsl - 1),
                        )
                ot = opool.tile([128, FI], FP32, tag="o")
                nc.scalar.copy(ot, ps)
                nc.sync.dma_start(
                    out=outT[jb * 128 : (jb + 1) * 128, :], in_=ot
                )
    nc.finalize()
    return nc


def _prep(feature_vectors, W1, b1, W2, b2):
    fv = np.asarray(feature_vectors, dtype=np.float32)
    W1 = np.asarray(W1, dtype=np.float32)
    b1 = np.asarray(b1, dtype=np.float32)
    W2 = np.asarray(W2, dtype=np.float32)
    b2 = np.asarray(b2, dtype=np.float32)

    L = fv @ W1[:, :F].T + b1        # [N, H], j side (bias, on partitions)
    R = fv @ W1[:, F:].T             # [N, H], i side (free dim)
    w2 = W2[0]                       # [H]
    b2v = float(b2[0])

    # Fold w2 into both operands; negative w2k handled with min-0 trick.
    Rs = R * w2[None, :]
    Ls = L * w2[None, :]

    min_ks = tuple(int(k) for k in range(H) if w2[k] < 0)
    act_ks, slices_spec = _plan(w2)
    nc = build_bass(min_ks, act_ks, slices_spec)

    ident = np.eye(128, dtype=np.float16)
    b2arr = np.full((128, 1), b2v, dtype=np.float32)
    in_maps = []
    for c in range(NCORES):
        ig, jg = divmod(c, JG)
        isl = slice(ig * FI, (ig + 1) * FI)
        jsl = slice(jg * PJ, (jg + 1) * PJ)
        base = Rs[isl, :].T.astype(np.float16)          # [H, FI]
        rb_c = np.ascontiguousarray(
            np.broadcast_to(base[:, None, :], (H, 128, FI))
        )
        lbt_c = np.ascontiguousarray(Ls[jsl, :].reshape(NJB, 128, H))
        in_maps.append(
            {"rb": rb_c, "lbt": lbt_c, "ident": ident, "b2t": b2arr}
        )
    return nc, in_maps


def _gather(res):
    out = np.empty((N, N), dtype=np.float32)
    for c in range(NCORES):
        ig, jg = divmod(c, JG)
        out[ig * FI : (ig + 1) * FI, jg * PJ : (jg + 1) * PJ] = (
            res.results[c]["outT"].T
        )
    return out


def kernel(feature_vectors, W1, b1, W2, b2):
    nc, in_maps = _prep(feature_vectors, W1, b1, W2, b2)
    res = run_bass_kernel_spmd(nc, in_maps, core_ids=list(range(NCORES)))
    return _gather(res)


# revision 12
# speedup vs baseline: 1.9714x; 1.0834x over previous
"""Trainium2 Bass kernel for NeuralConnectionMatrix.

out[i, j] = W2 . relu(R[i, :] + L[j, :] + b1) + b2
  where L = fv @ W1[:, :F].T  (depends on j), R = fv @ W1[:, F:].T (depends on i)

Sharding (8 cores): 2 i-groups x 4 j-groups. Each core computes a
[1024 j, 2048 i] transposed slab:
  - partitions = j (8 blocks of 128), free dim = i (2048)
  - per k: t_k = w2k*relu(x_k) computed as (w2k*R_bcast + w2k*Lb_bias) max/min 0
    (min-0 trick bakes negative w2k signs in host-side); relus split
    across VectorE (tensor_scalar) and ScalarE (activation relu)
  - pairs of t_k are pre-summed on GpSimd/VectorE (b2 folded into the
    first merge) to cut TensorE matmul count
  - PE accumulates the resulting slices into PSUM via identity matmuls
  - ACT drains PSUM -> SBUF, DMA to DRAM
Host precomputes L/R (tiny GEMMs), replicates R across partitions, and
transposes the per-core output slabs back into the full [4096, 4096] array.
"""

import numpy as np

import concourse.bass as bass
import concourse.bacc as bacc
import concourse.mybir as mybir
from concourse.tile import TileContext
from concourse.bass_utils import run_bass_kernel_spmd

N = 4096
F = 3
H = 16
NCORES = 8
IG, JG = 2, 4            # core grid over (i, j)
FI = N // IG             # free-dim (i) extent per core: 2048
PJ = N // JG             # partition-dim (j) extent per core: 1024
NJB = PJ // 128          # j blocks per core: 8
NMM = FI // 512          # matmuls per slice per j-block (PSUM bank = 512 f32)

FP16 = mybir.dt.float16
FP32 = mybir.dt.float32
ALU = mybir.AluOpType


def _plan(w2, act_n=4, n_pairs=3):
    """Assign the 16 k-slices to engines and merge pairs.

    Returns (act_ks, slices_spec): entries ("s", k) or
    ("p", ka, kb, fold_b2); pairs run on DVE (gpsimd 2-input ops are
    slow, ScalarE can't). b2 folds into the first pair via
    scalar_tensor_tensor. Ordered roughly by production.
    """
    pos = [int(k) for k in range(H) if w2[k] >= 0]
    neg = [int(k) for k in range(H) if w2[k] < 0]
    act_n = min(act_n, len(pos))
    act_ks = pos[:act_n]
    d = pos[act_n:] + neg
    a = list(act_ks)

    pairs = []
    while len(pairs) < n_pairs and len(d) >= 2:
        pairs.append((d.pop(0), d.pop(0)))

    slices_spec = []
    # interleave ACT singles among DVE singles; pairs in the middle
    mix = []
    di, ai = 0, 0
    for k in range(len(d) + len(a)):
        if ai < len(a) and (di >= len(d) or k % 3 == 2):
            mix.append(("s", a[ai])); ai += 1
        else:
            mix.append(("s", d[di])); di += 1
    mid = len(mix) // 2
    slices_spec = mix[:mid]
    for pi, (ka, kb) in enumerate(pairs):
        slices_spec.append(("p", ka, kb, pi == 0))
    slices_spec += mix[mid:]
    if not pairs:
        raise ValueError("need at least one pair to fold b2")
    return act_ks, slices_spec


def _rb_order(slices_spec):
    """k indices in consumption order (for DMA issue order)."""
    order = []
    for spec in slices_spec:
        if spec[0] == "s":
            order.append(spec[1])
        else:
            order.extend([spec[1], spec[2]])
    return order


def build_bass(min_ks, act_ks, slices_spec):
    nc = bacc.Bacc()
    rb = nc.dram_tensor("rb", [H, 128, FI], FP16, kind="ExternalInput")
    lbt = nc.dram_tensor("lbt", [NJB, 128, H], FP32, kind="ExternalInput")
    ident = nc.dram_tensor("ident", [128, 128], FP16, kind="ExternalInput")
    b2t = nc.dram_tensor("b2t", [128, 1], FP32, kind="ExternalInput")
    outT = nc.dram_tensor("outT", [PJ, FI], FP32, kind="ExternalOutput")

    with TileContext(nc) as tc:
        with (
            tc.tile_pool(name="const", bufs=1) as cpool,
            tc.tile_pool(name="t", bufs=10) as tpool,
            tc.tile_pool(name="u", bufs=6) as upool,
            tc.tile_pool(name="o", bufs=2) as opool,
            tc.tile_pool(name="ps", bufs=2, space="PSUM") as pspool,
        ):
            id_t = cpool.tile([128, 128], FP16, tag="ident")
            nc.gpsimd.dma_start(out=id_t, in_=ident[:, :])
            lb_all = cpool.tile([128, NJB * H], FP32, tag="lball")
            nc.gpsimd.dma_start(
                out=lb_all.rearrange("p (b k) -> p b k", b=NJB),
                in_=lbt.rearrange("b p k -> p b k"),
            )
            b2_sb = cpool.tile([128, 1], FP32, tag="b2")
            nc.gpsimd.dma_start(out=b2_sb, in_=b2t[:, :])
            rbs = {}
            for ki, k in enumerate(_rb_order(slices_spec)):
                rt = cpool.tile([128, FI], FP16, tag=f"rb{k}")
                dma_eng = nc.sync if ki < 2 else nc.gpsimd
                dma_eng.dma_start(out=rt, in_=rb[k])
                rbs[k] = rt

            for jb in range(NJB):
                ps = pspool.tile([128, FI], FP32, tag="ps")

                def relu(k):
                    t = tpool.tile([128, FI], FP16, tag="t")
                    lb_ap = lb_all[:, jb * H + k : jb * H + k + 1]
                    if k in act_ks:
                        nc.scalar.activation(
                            t, rbs[k], mybir.ActivationFunctionType.Relu,
                            bias=lb_ap, scale=1.0,
                        )
                    else:
                        op1 = ALU.min if k in min_ks else ALU.max
                        nc.vector.tensor_scalar(
                            out=t, in0=rbs[k],
                            scalar1=lb_ap, scalar2=0.0,
                            op0=ALU.add, op1=op1,
                        )
                    return t

                nsl = len(slices_spec)
                for si, spec in enumerate(slices_spec):
                    if spec[0] == "s":
                        sl = relu(spec[1])
                    else:
                        _, ka, kb, _fold = spec
                        ta, tb = relu(ka), relu(kb)
                        sl = upool.tile([128, FI], FP16, tag="u")
                        nc.vector.tensor_add(out=sl, in0=ta, in1=tb)
                    for nb in range(NMM):
                        nc.tensor.matmul(
                            ps[:, nb * 512 : (nb + 1) * 512],
                            id_t,
                            sl[:, nb * 512 : (nb + 1) * 512],
                            start=(si == 0), stop=(si == nsl - 1),
                        )
                ot = opool.tile([128, FI], FP32, tag="o")
                # drain PSUM -> SBUF; the +b2 rides along as the affine bias
                nc.scalar.activation(
                    ot, ps, mybir.ActivationFunctionType.Identity,
                    bias=b2_sb[:, 0:1], scale=1.0,
                )
                nc.sync.dma_start(
                    out=outT[jb * 128 : (jb + 1) * 128, :], in_=ot
                )
    nc.finalize()
    return nc


def _prep(feature_vectors, W1, b1, W2, b2):
    fv = np.asarray(feature_vectors, dtype=np.float32)
    W1 = np.asarray(W1, dtype=np.float32)
    b1 = np.asarray(b1, dtype=np.float32)
    W2 = np.asarray(W2, dtype=np.float32)
    b2 = np.asarray(b2, dtype=np.float32)

    L = fv @ W1[:, :F].T + b1        # [N, H], j side (bias, on partitions)
    R = fv @ W1[:, F:].T             # [N, H], i side (free dim)
    w2 = W2[0]                       # [H]
    b2v = float(b2[0])

    # Fold w2 into both operands; negative w2k handled with min-0 trick.
    Rs = R * w2[None, :]
    Ls = L * w2[None, :]

    min_ks = tuple(int(k) for k in range(H) if w2[k] < 0)
    act_ks, slices_spec = _plan(w2)
    nc = build_bass(min_ks, act_ks, slices_spec)

    ident = np.eye(128, dtype=np.float16)
    in_maps = []
    for c in range(NCORES):
        ig, jg = divmod(c, JG)
        isl = slice(ig * FI, (ig + 1) * FI)
        jsl = slice(jg * PJ, (jg + 1) * PJ)
        base = Rs[isl, :].T.astype(np.float16)          # [H, FI]
        rb_c = np.ascontiguousarray(
            np.broadcast_to(base[:, None, :], (H, 128, FI))
        )
        lbt_c = np.ascontiguousarray(Ls[jsl, :].reshape(NJB, 128, H))
        in_maps.append(
            {"rb": rb_c, "lbt": lbt_c, "ident": ident,
             "b2t": np.full((128, 1), b2v, dtype=np.float32)}
        )
    return nc, in_maps


def _gather(res):
    out = np.empty((N, N), dtype=np.float32)
    for c in range(NCORES):
        ig, jg = divmod(c, JG)
        out[ig * FI : (ig + 1) * FI, jg * PJ : (jg + 1) * PJ] = (
            res.results[c]["outT"].T
        )
    return out


def kernel(feature_vectors, W1, b1, W2, b2):
    nc, in_maps = _prep(feature_vectors, W1, b1, W2, b2)
    res = run_bass_kernel_spmd(nc, in_maps, core_ids=list(range(NCORES)))
    return _gather(res)
